# revision 11
# baseline (speedup 1.0000x reference)
"""Trainium2 Bass kernel for decayed event scatter-add (ExtractExclusivePatches).

Computes, for E events with sorted segment ids:
    out[n, k, c] = sum_{e: seg_e = n, kid_e = k} f_e[c] * exp(-(t_out[n] - dt_e) * rate_c)
with rate = softplus(decay_rate), out shape [N_OUT, K, C].

Strategy (8 NeuronCores, SPMD, no collectives):
  - Each core owns a contiguous range of output segments (N_OUT/8), i.e. a
    contiguous range of "flat slots" (flat = seg*K + kid, 225000 slots/core).
  - Host bins events by flat slot into per-core 128-slot windows and pads each
    window's event list to a fixed Kpad (uniform program across cores).
  - Device, per window: one DVE tensor_scalar builds a scaled one-hot matrix
    [Kpad events, 128 slots] = (iota == off) * g  where g = exp(-rate*elapsed)
    is the per-event decay (ACT engine); one matmul scatters the raw feature
    rows into a PSUM tile [128 slots, 64 ch]; ACT copies PSUM->SBUF staging;
    one DMA writes 8 windows (1024 slots) of contiguous output rows.
  - If rate is not channel-constant (decay_rate not constant), a general path
    computes per-event-per-channel decay on ACT and multiplies features on DVE.
"""

import math
import os

import numpy as np

# ---- problem constants (hardcoded per contract) ----
E_IN = 1_000_000
N_OUT = 200_000
C = 64
K = 9
NCORES = 8

SEGS_PER_CORE = N_OUT // NCORES          # 25000
SLOTS_PER_CORE = SEGS_PER_CORE * K       # 225000
W = 128                                   # slots per window (matmul M)
WPG = 8                                   # windows per group (psum banks / staging)
GROUPS = math.ceil(SLOTS_PER_CORE / (W * WPG))   # 220
WINDOWS = GROUPS * WPG                    # 1760
SGR = 8                                   # groups per scal DMA (64 windows)
SGROUPS = math.ceil(GROUPS / SGR)         # 28

_LN2 = float(np.log(2.0))


def _softplus(x):
    return np.logaddexp(0.0, x)


# ---------------------------------------------------------------- host side


def _preprocess(features, dt, times_out, successor_kernel_ids, segment_ids_out,
                decay_rate):
    """Bin events into per-core per-window padded streams."""
    seg = np.asarray(segment_ids_out, dtype=np.int64)
    kid = np.asarray(successor_kernel_ids, dtype=np.int64)
    flat = seg * K + kid                                    # [E] in [0, N_OUT*K)
    elapsed = (np.asarray(times_out, dtype=np.float32)[seg]
               - np.asarray(dt, dtype=np.float32))          # [E]

    core = flat // SLOTS_PER_CORE                           # [E] in [0,8)
    local = flat - core * SLOTS_PER_CORE
    w_local = local // W                                    # [0, 1758)
    off = (local - w_local * W).astype(np.float32)          # [0, 128)

    gw = core * WINDOWS + w_local                           # global window id
    order = np.argsort(gw, kind="stable")
    gw_s = gw[order]
    counts = np.bincount(gw_s, minlength=NCORES * WINDOWS)
    starts = np.concatenate([[0], np.cumsum(counts)[:-1]])
    rank = np.arange(E_IN, dtype=np.int64) - starts[gw_s]

    kpad = int(counts.max())
    assert kpad <= 128, f"window overflow: {kpad} events in one 128-slot window"
    # round up a little for DMA friendliness
    kpad = min(128, ((kpad + 3) // 4) * 4)

    # reorder per-event streams into sorted (core, window) order
    core_s = core[order]
    w_local_s = w_local[order]
    off_s = off[order]
    elapsed_s = elapsed[order]
    grp_s = w_local_s // WPG
    sub_s = w_local_s - grp_s * WPG

    # padded per-(core,window) feature stream: [NC, GROUPS, kpad, WPG, C]
    featw = np.zeros((NCORES * GROUPS * kpad * WPG, C), dtype=np.float32)
    dest = ((core_s * GROUPS + grp_s) * kpad + rank) * WPG + sub_s
    featw[dest] = np.asarray(features, dtype=np.float32)[order]
    featw = featw.reshape(NCORES, GROUPS, kpad, WPG * C)

    # scal stream: [NC, SGROUPS, kpad, SGR, 2, WPG]; j=0 -> elapsed, j=1 -> off
    scal = np.zeros((NCORES, SGROUPS, kpad, SGR, 2, WPG), dtype=np.float32)
    scal[:, :, :, :, 1, :] = -1.0                          # off=-1 -> no match
    sgrp_s = grp_s // SGR
    gg_s = grp_s - sgrp_s * SGR
    sdest = (((core_s * SGROUPS + sgrp_s) * kpad + rank) * SGR + gg_s) * 2 * WPG
    scal_flat = scal.reshape(-1)
    scal_flat[sdest + sub_s] = elapsed_s
    scal_flat[sdest + WPG + sub_s] = off_s
    scal = scal_flat.reshape(NCORES, SGROUPS, kpad, SGR * 2 * WPG)

    iota = np.tile(np.arange(W, dtype=np.float32), (128, 1))
    return featw, scal, iota, kpad


def _build_program(kpad, rate, groups=GROUPS, sgroups=SGROUPS, slots=None,
                   lhst_dt="bfloat16", rhs_dt="bfloat16", gp_split=0,
                   pack_psum=True):
    """Build the Bass/Tile program (uniform across cores).

    lhst_dt: dtype of the one-hot (matmul stationary operand).
    rhs_dt: dtype of the feature stream (matmul moving operand).
    gp_split: every gp_split-th one-hot build goes to GpSimd (0 = all DVE).
    pack_psum: pack a group's 8 windows into one PSUM bank (single flush).
    """
    import concourse.bacc as bacc
    import concourse.mybir as mybir
    import concourse.tile as tile

    rate = np.asarray(rate, dtype=np.float32)
    const_rate = bool(np.ptp(rate) <= 1e-12 * max(1.0, abs(float(rate[0]))))
    if slots is None:
        slots = groups * W * WPG
    lhst_mdt = getattr(mybir.dt, lhst_dt)
    rhs_mdt = getattr(mybir.dt, rhs_dt)
    # fp32 moving operand legally requires fp32 stationary (and vice versa)
    onehot_mdt = lhst_mdt if lhst_dt != "float32r" else mybir.dt.float32

    nc = bacc.Bacc("TRN2", target_bir_lowering=False, debug=False,
                   enable_asserts=False)

    featw_d = nc.dram_tensor("featw", [groups, kpad, WPG * C], rhs_mdt,
                             kind="ExternalInput")
    scal_d = nc.dram_tensor("scal", [sgroups, kpad, SGR * 2 * WPG],
                            mybir.dt.float32, kind="ExternalInput")
    iota_d = nc.dram_tensor("iota", [128, W], mybir.dt.float32,
                            kind="ExternalInput")
    ratebc_d = None
    if not const_rate:
        ratebc_d = nc.dram_tensor("ratebc", [128, C], mybir.dt.float32,
                                  kind="ExternalInput")
    out_d = nc.dram_tensor("out", [slots, C], mybir.dt.float32,
                           kind="ExternalOutput")

    with tile.TileContext(nc) as tc:
        with (
            tc.tile_pool(name="const", bufs=1) as constp,
            tc.tile_pool(name="feats", bufs=6) as featp,
            tc.tile_pool(name="scal", bufs=3) as scalp,
            tc.tile_pool(name="work", bufs=10) as workp,
            tc.tile_pool(name="stage", bufs=6) as stagep,
            tc.tile_pool(name="psum", bufs=8, space="PSUM") as psump,
        ):
            iota_t = constp.tile([128, W], onehot_mdt)
            nc.gpsimd.dma_start(out=iota_t[:], in_=iota_d.ap())
            ratebc_t = None
            if not const_rate:
                ratebc_t = constp.tile([128, C], mybir.dt.float32)
                nc.sync.dma_start(out=ratebc_t[:], in_=ratebc_d.ap())

            def fetch_sgroup(sg):
                """DMA one scal group and compute its decay factors."""
                scal_t = scalp.tile([kpad, SGR * 2 * WPG], mybir.dt.float32,
                                    name=f"scal_{sg}", tag="scal")
                nc.sync.dma_start(out=scal_t[:], in_=scal_d.ap()[sg])
                scal_v = scal_t[:].rearrange("p (g j w) -> p g j w", g=SGR, j=2)
                g_t = None
                if const_rate:
                    # g[e] = exp(-rate0 * elapsed[e]) for 64 windows at once
                    g_t = workp.tile([kpad, SGR * WPG], mybir.dt.float32,
                                     name=f"gdecay_{sg}", tag="gdecay", bufs=3)
                    nc.scalar.activation(
                        out=g_t[:].rearrange("p (g w) -> p g w", g=SGR),
                        in_=scal_v[:, :, 0, :],
                        func=mybir.ActivationFunctionType.Exp,
                        scale=-float(rate[0]),
                    )
                return scal_v, g_t

            widx = 0
            pref = {0: fetch_sgroup(0)}
            if sgroups > 1:
                pref[1] = fetch_sgroup(1)
            for sg in range(sgroups):
                scal_v, g_t = pref.pop(sg)
                if sg + 2 < sgroups:
                    pref[sg + 2] = fetch_sgroup(sg + 2)

                for gg in range(min(SGR, groups - sg * SGR)):
                    grp = sg * SGR + gg
                    feat_eng = nc.sync if grp % 2 == 0 else nc.scalar
                    feat_t = featp.tile([kpad, WPG * C], rhs_mdt)
                    feat_eng.dma_start(out=feat_t[:], in_=featw_d.ap()[grp])
                    stage_t = stagep.tile([128, WPG * C], mybir.dt.float32)
                    if pack_psum:
                        psum_t = psump.tile([128, WPG * C], mybir.dt.float32,
                                            tag="acc")

                    for w in range(WPG):
                        off_col = scal_v[:, gg, 1, w:w + 1]
                        onehot_t = workp.tile([kpad, W], onehot_mdt,
                                              tag="onehot")
                        widx += 1
                        eng = (nc.gpsimd if (gp_split and widx % gp_split == 0)
                               else nc.vector)
                        if const_rate:
                            eng.tensor_scalar(
                                out=onehot_t[:],
                                in0=iota_t[:kpad, :],
                                scalar1=off_col,
                                scalar2=g_t[:, gg * WPG + w:gg * WPG + w + 1],
                                op0=mybir.AluOpType.is_equal,
                                op1=mybir.AluOpType.mult,
                            )
                            rhs = feat_t[:].rearrange(
                                "p (w c) -> p w c", w=WPG)[:, w, :]
                        else:
                            eng.tensor_scalar(
                                out=onehot_t[:],
                                in0=iota_t[:kpad, :],
                                scalar1=off_col,
                                scalar2=None,
                                op0=mybir.AluOpType.is_equal,
                            )
                            decay_t = workp.tile([kpad, C], mybir.dt.float32,
                                                 tag="decay")
                            nc.scalar.activation(
                                out=decay_t[:],
                                in_=ratebc_t[:kpad, :],
                                func=mybir.ActivationFunctionType.Exp,
                                scale=scal_v[:, gg, 0, w:w + 1],
                            )
                            vals_t = workp.tile([kpad, C], rhs_mdt,
                                                tag="vals")
                            nc.vector.tensor_tensor(
                                out=vals_t[:],
                                in0=feat_t[:].rearrange(
                                    "p (w c) -> p w c", w=WPG)[:, w, :],
                                in1=decay_t[:],
                                op=mybir.AluOpType.mult,
                            )
                            rhs = vals_t[:]

                        lhsT = onehot_t[:]
                        if lhst_dt == "float32r":
                            lhsT = lhsT.bitcast(mybir.dt.float32r)
                        if pack_psum:
                            nc.tensor.matmul(
                                out=psum_t[:, w * C:(w + 1) * C],
                                lhsT=lhsT,
                                rhs=rhs,
                                start=(w == 0),
                                stop=(w == WPG - 1),
                                skip_group_check=True,
                            )
                        else:
                            psum_t = psump.tile([128, C], mybir.dt.float32,
                                                tag="acc")
                            nc.tensor.matmul(
                                out=psum_t[:], lhsT=lhsT, rhs=rhs,
                                start=True, stop=True,
                            )
                            nc.scalar.copy(
                                out=stage_t[:, w * C:(w + 1) * C],
                                in_=psum_t[:])

                    if pack_psum:
                        nc.scalar.copy(out=stage_t[:], in_=psum_t[:])
                    out_eng = nc.scalar if grp % 2 == 0 else nc.sync
                    out_eng.dma_start(
                        out=out_d.ap()[grp * W * WPG:(grp + 1) * W * WPG]
                        .rearrange("(w p) c -> p w c", p=128),
                        in_=stage_t[:].rearrange("p (w c) -> p w c", w=WPG),
                    )
    nc.compile()
    return nc


def _run(nc, in_maps, **kwargs):
    from concourse import bass_utils
    return bass_utils.run_bass_kernel_spmd(
        nc, in_maps, core_ids=list(range(len(in_maps))), **kwargs)


DEFAULT_CFG = {
    "lhst_dt": "bfloat16",
    "rhs_dt": "bfloat16",
    "gp_split": 0,
    "pack_psum": True,
}


def kernel(features, dt, times_out, successor_kernel_ids, segment_ids_out,
           decay_rate, _bench=None, _cfg=None):
    import ml_dtypes

    cfg = dict(DEFAULT_CFG, **(_cfg or {}))
    features = np.asarray(features, dtype=np.float32)
    rate = _softplus(np.asarray(decay_rate, dtype=np.float32))

    featw, scal, iota, kpad = _preprocess(
        features, dt, times_out, successor_kernel_ids, segment_ids_out,
        decay_rate)
    if cfg["rhs_dt"] == "bfloat16":
        featw = featw.astype(ml_dtypes.bfloat16)

    nc = _build_program(kpad, rate, **cfg)

    const_rate = bool(np.ptp(rate) <= 1e-12 * max(1.0, abs(float(rate[0]))))
    in_maps = []
    for c in range(NCORES):
        m = {"featw": featw[c], "scal": scal[c], "iota": iota}
        if not const_rate:
            m["ratebc"] = np.tile(-rate, (128, 1)).astype(np.float32)
        in_maps.append(m)

    if _bench is not None:
        res = _run(nc, in_maps, **_bench)
        outs = [r["out"] for r in res.results]
        full = np.concatenate([o[:SLOTS_PER_CORE] for o in outs], axis=0)
        return full.reshape(N_OUT, K, C), res

    res = _run(nc, in_maps)
    outs = [r["out"] for r in res.results]
    full = np.concatenate([o[:SLOTS_PER_CORE] for o in outs], axis=0)
    return full.reshape(N_OUT, K, C)


# revision 19
# speedup vs baseline: 1.0154x; 1.0154x over previous
"""Trainium2 Bass kernel for decayed event scatter-add (ExtractExclusivePatches).

Computes, for E events with sorted segment ids:
    out[n, k, c] = sum_{e: seg_e = n, kid_e = k} f_e[c] * exp(-(t_out[n] - dt_e) * rate_c)
with rate = softplus(decay_rate), out shape [N_OUT, K, C].

Strategy (8 NeuronCores, SPMD, no collectives):
  - Each core owns a contiguous range of output segments (N_OUT/8), i.e. a
    contiguous range of "flat slots" (flat = seg*K + kid, 225000 slots/core).
  - Host bins events by flat slot into per-core 128-slot windows and pads each
    window's event list to a fixed Kpad (uniform program across cores).
  - Device, per window: one DVE tensor_scalar builds a scaled one-hot matrix
    [Kpad events, 128 slots] = (iota == off) * g  where g = exp(-rate*elapsed)
    is the per-event decay (ACT engine); one matmul scatters the raw feature
    rows into a PSUM tile [128 slots, 64 ch]; ACT copies PSUM->SBUF staging;
    one DMA writes 8 windows (1024 slots) of contiguous output rows.
  - If rate is not channel-constant (decay_rate not constant), a general path
    computes per-event-per-channel decay on ACT and multiplies features on DVE.
"""

import math
import os

import numpy as np

# ---- problem constants (hardcoded per contract) ----
E_IN = 1_000_000
N_OUT = 200_000
C = 64
K = 9
NCORES = 8

SEGS_PER_CORE = N_OUT // NCORES          # 25000
SLOTS_PER_CORE = SEGS_PER_CORE * K       # 225000
W = 128                                   # slots per window (matmul M)
WPG = 8                                   # windows per group (psum banks / staging)
GROUPS = math.ceil(SLOTS_PER_CORE / (W * WPG))   # 220
WINDOWS = GROUPS * WPG                    # 1760
SGR = 8                                   # groups per scal DMA (64 windows)
SGROUPS = math.ceil(GROUPS / SGR)         # 28

_LN2 = float(np.log(2.0))


def _softplus(x):
    return np.logaddexp(0.0, x)


# ---------------------------------------------------------------- host side


def _preprocess(features, dt, times_out, successor_kernel_ids, segment_ids_out,
                decay_rate):
    """Bin events into per-core per-window padded streams.

    scal field j=0 holds ln(g) = -rate0*elapsed when rate is channel-constant
    (else raw elapsed); j=1 holds -off (negated slot offset).
    """
    rate = _softplus(np.asarray(decay_rate, dtype=np.float32))
    const_rate = bool(np.ptp(rate) <= 1e-12 * max(1.0, abs(float(rate[0]))))
    seg = np.asarray(segment_ids_out, dtype=np.int64)
    kid = np.asarray(successor_kernel_ids, dtype=np.int64)
    flat = seg * K + kid                                    # [E] in [0, N_OUT*K)
    elapsed = (np.asarray(times_out, dtype=np.float32)[seg]
               - np.asarray(dt, dtype=np.float32))          # [E]
    if const_rate:
        elapsed = -float(rate[0]) * elapsed                 # = ln(g)

    core = flat // SLOTS_PER_CORE                           # [E] in [0,8)
    local = flat - core * SLOTS_PER_CORE
    w_local = local // W                                    # [0, 1758)
    off = (local - w_local * W).astype(np.float32)          # [0, 128)

    gw = core * WINDOWS + w_local                           # global window id
    order = np.argsort(gw, kind="stable")
    gw_s = gw[order]
    counts = np.bincount(gw_s, minlength=NCORES * WINDOWS)
    starts = np.concatenate([[0], np.cumsum(counts)[:-1]])
    rank = np.arange(E_IN, dtype=np.int64) - starts[gw_s]

    kpad = int(counts.max())
    assert kpad <= 128, f"window overflow: {kpad} events in one 128-slot window"
    # round up a little for DMA friendliness
    kpad = min(128, ((kpad + 3) // 4) * 4)

    # reorder per-event streams into sorted (core, window) order
    core_s = core[order]
    w_local_s = w_local[order]
    off_s = off[order]
    elapsed_s = elapsed[order]
    grp_s = w_local_s // WPG
    sub_s = w_local_s - grp_s * WPG

    # padded per-(core,window) feature stream: [NC, GROUPS, kpad, WPG, C]
    featw = np.zeros((NCORES * GROUPS * kpad * WPG, C), dtype=np.float32)
    dest = ((core_s * GROUPS + grp_s) * kpad + rank) * WPG + sub_s
    featw[dest] = np.asarray(features, dtype=np.float32)[order]
    featw = featw.reshape(NCORES, GROUPS, kpad, WPG * C)

    # scal stream: [NC, SGROUPS, kpad, SGR, 2, WPG]; j=0 -> ln(g), j=1 -> -off
    scal = np.zeros((NCORES, SGROUPS, kpad, SGR, 2, WPG), dtype=np.float32)
    scal[:, :, :, :, 1, :] = 1.0                           # -off=1 -> no match
    sgrp_s = grp_s // SGR
    gg_s = grp_s - sgrp_s * SGR
    sdest = (((core_s * SGROUPS + sgrp_s) * kpad + rank) * SGR + gg_s) * 2 * WPG
    scal_flat = scal.reshape(-1)
    scal_flat[sdest + sub_s] = elapsed_s
    scal_flat[sdest + WPG + sub_s] = -off_s
    scal = scal_flat.reshape(NCORES, SGROUPS, kpad, SGR * 2 * WPG)

    # iotas[0] = +col (ACT path), iotas[1] = -col (DVE is_equal vs -off)
    iota = np.stack([np.tile(np.arange(W, dtype=np.float32), (128, 1)),
                     np.tile(-np.arange(W, dtype=np.float32), (128, 1))])
    return featw, scal, iota, kpad


def _build_program(kpad, rate, groups=GROUPS, sgroups=SGROUPS, slots=None,
                   lhst_dt="bfloat16", rhs_dt="bfloat16", gp_split=0,
                   act_split=0, pack_psum=True):
    """Build the Bass/Tile program (uniform across cores).

    lhst_dt: dtype of the one-hot (matmul stationary operand).
    rhs_dt: dtype of the feature stream (matmul moving operand).
    gp_split: every gp_split-th one-hot build goes to GpSimd (0 = all DVE).
    act_split: every act_split-th one-hot build goes to ScalarE via the
        Square -> Exp(-90 t + ln g) trick (0 = none; const-rate only).
    pack_psum: pack a group's 8 windows into one PSUM bank (single flush).
    """
    import concourse.bacc as bacc
    import concourse.mybir as mybir
    import concourse.tile as tile

    rate = np.asarray(rate, dtype=np.float32)
    const_rate = bool(np.ptp(rate) <= 1e-12 * max(1.0, abs(float(rate[0]))))
    if slots is None:
        slots = groups * W * WPG
    lhst_mdt = getattr(mybir.dt, lhst_dt)
    rhs_mdt = getattr(mybir.dt, rhs_dt)
    # fp32 moving operand legally requires fp32 stationary (and vice versa)
    onehot_mdt = lhst_mdt if lhst_dt != "float32r" else mybir.dt.float32

    nc = bacc.Bacc("TRN2", target_bir_lowering=False, debug=False,
                   enable_asserts=False)

    featw_d = nc.dram_tensor("featw", [groups, kpad, WPG * C], rhs_mdt,
                             kind="ExternalInput")
    scal_d = nc.dram_tensor("scal", [sgroups, kpad, SGR * 2 * WPG],
                            mybir.dt.float32, kind="ExternalInput")
    iota_d = nc.dram_tensor("iota", [2, 128, W], mybir.dt.float32,
                            kind="ExternalInput")
    ratebc_d = None
    if not const_rate:
        ratebc_d = nc.dram_tensor("ratebc", [128, C], mybir.dt.float32,
                                  kind="ExternalInput")
    out_d = nc.dram_tensor("out", [slots, C], mybir.dt.float32,
                           kind="ExternalOutput")

    with tile.TileContext(nc) as tc:
        with (
            tc.tile_pool(name="const", bufs=1) as constp,
            tc.tile_pool(name="feats", bufs=6) as featp,
            tc.tile_pool(name="scal", bufs=3) as scalp,
            tc.tile_pool(name="work", bufs=10) as workp,
            tc.tile_pool(name="stage", bufs=6) as stagep,
            tc.tile_pool(name="psum", bufs=8, space="PSUM") as psump,
        ):
            iota_pos_t = constp.tile([128, W], mybir.dt.float32)
            nc.gpsimd.dma_start(out=iota_pos_t[:], in_=iota_d.ap()[0])
            iota_t = constp.tile([128, W], onehot_mdt)
            nc.gpsimd.dma_start(out=iota_t[:], in_=iota_d.ap()[1])
            ratebc_t = None
            if not const_rate:
                ratebc_t = constp.tile([128, C], mybir.dt.float32)
                nc.sync.dma_start(out=ratebc_t[:], in_=ratebc_d.ap())

            def fetch_sgroup(sg):
                """DMA one scal group and compute its decay factors."""
                scal_t = scalp.tile([kpad, SGR * 2 * WPG], mybir.dt.float32,
                                    name=f"scal_{sg}", tag="scal")
                nc.sync.dma_start(out=scal_t[:], in_=scal_d.ap()[sg])
                scal_v = scal_t[:].rearrange("p (g j w) -> p g j w", g=SGR, j=2)
                g_t = None
                if const_rate:
                    # g[e] = exp(-rate0 * elapsed[e]) for 64 windows at once
                    g_t = workp.tile([kpad, SGR * WPG], mybir.dt.float32,
                                     name=f"gdecay_{sg}", tag="gdecay", bufs=3)
                    nc.scalar.activation(
                        out=g_t[:].rearrange("p (g w) -> p g w", g=SGR),
                        in_=scal_v[:, :, 0, :],
                        func=mybir.ActivationFunctionType.Exp,
                        scale=1.0,
                    )
                return scal_v, g_t

            widx = 0
            pref = {0: fetch_sgroup(0)}
            if sgroups > 1:
                pref[1] = fetch_sgroup(1)
            for sg in range(sgroups):
                scal_v, g_t = pref.pop(sg)
                if sg + 2 < sgroups:
                    pref[sg + 2] = fetch_sgroup(sg + 2)

                for gg in range(min(SGR, groups - sg * SGR)):
                    grp = sg * SGR + gg
                    feat_eng = nc.sync if grp % 2 == 0 else nc.scalar
                    feat_t = featp.tile([kpad, WPG * C], rhs_mdt)
                    feat_eng.dma_start(out=feat_t[:], in_=featw_d.ap()[grp])
                    stage_t = stagep.tile([128, WPG * C], mybir.dt.float32)
                    if pack_psum:
                        psum_t = psump.tile([128, WPG * C], mybir.dt.float32,
                                            tag="acc")

                    for w in range(WPG):
                        off_col = scal_v[:, gg, 1, w:w + 1]
                        onehot_t = workp.tile([kpad, W], onehot_mdt,
                                              tag="onehot")
                        widx += 1
                        eng = (nc.gpsimd if (gp_split and widx % gp_split == 0)
                               else nc.vector)
                        use_act = (act_split and const_rate
                                   and widx % act_split == 0)
                        if use_act:
                            # onehot*g = exp(-90*(iota-off)^2 + ln g), exact
                            # for integer iota/off (0 or g).
                            sq_t = workp.tile([kpad, W], mybir.dt.float32,
                                              tag="actsq")
                            nc.scalar.activation(
                                out=sq_t[:],
                                in_=iota_pos_t[:kpad, :],
                                func=mybir.ActivationFunctionType.Square,
                                scale=1.0,
                                bias=off_col,
                            )
                            nc.scalar.activation(
                                out=onehot_t[:],
                                in_=sq_t[:],
                                func=mybir.ActivationFunctionType.Exp,
                                scale=-90.0,
                                bias=scal_v[:, gg, 0, w:w + 1],
                            )
                            rhs = feat_t[:].rearrange(
                                "p (w c) -> p w c", w=WPG)[:, w, :]
                        elif const_rate:
                            eng.tensor_scalar(
                                out=onehot_t[:],
                                in0=iota_t[:kpad, :],
                                scalar1=off_col,
                                scalar2=g_t[:, gg * WPG + w:gg * WPG + w + 1],
                                op0=mybir.AluOpType.is_equal,
                                op1=mybir.AluOpType.mult,
                            )
                            rhs = feat_t[:].rearrange(
                                "p (w c) -> p w c", w=WPG)[:, w, :]
                        else:
                            eng.tensor_scalar(
                                out=onehot_t[:],
                                in0=iota_t[:kpad, :],
                                scalar1=off_col,
                                scalar2=None,
                                op0=mybir.AluOpType.is_equal,
                            )
                            decay_t = workp.tile([kpad, C], mybir.dt.float32,
                                                 tag="decay")
                            nc.scalar.activation(
                                out=decay_t[:],
                                in_=ratebc_t[:kpad, :],
                                func=mybir.ActivationFunctionType.Exp,
                                scale=scal_v[:, gg, 0, w:w + 1],
                            )
                            vals_t = workp.tile([kpad, C], rhs_mdt,
                                                tag="vals")
                            nc.vector.tensor_tensor(
                                out=vals_t[:],
                                in0=feat_t[:].rearrange(
                                    "p (w c) -> p w c", w=WPG)[:, w, :],
                                in1=decay_t[:],
                                op=mybir.AluOpType.mult,
                            )
                            rhs = vals_t[:]

                        lhsT = onehot_t[:]
                        if lhst_dt == "float32r":
                            lhsT = lhsT.bitcast(mybir.dt.float32r)
                        if pack_psum:
                            nc.tensor.matmul(
                                out=psum_t[:, w * C:(w + 1) * C],
                                lhsT=lhsT,
                                rhs=rhs,
                                start=(w == 0),
                                stop=(w == WPG - 1),
                                skip_group_check=True,
                            )
                        else:
                            psum_t = psump.tile([128, C], mybir.dt.float32,
                                                tag="acc")
                            nc.tensor.matmul(
                                out=psum_t[:], lhsT=lhsT, rhs=rhs,
                                start=True, stop=True,
                            )
                            nc.scalar.copy(
                                out=stage_t[:, w * C:(w + 1) * C],
                                in_=psum_t[:])

                    if pack_psum:
                        nc.scalar.copy(out=stage_t[:], in_=psum_t[:])
                    out_eng = nc.scalar if grp % 2 == 0 else nc.sync
                    out_eng.dma_start(
                        out=out_d.ap()[grp * W * WPG:(grp + 1) * W * WPG]
                        .rearrange("(w p) c -> p w c", p=128),
                        in_=stage_t[:].rearrange("p (w c) -> p w c", w=WPG),
                    )
    nc.compile()
    return nc


def _run(nc, in_maps, **kwargs):
    from concourse import bass_utils
    return bass_utils.run_bass_kernel_spmd(
        nc, in_maps, core_ids=list(range(len(in_maps))), **kwargs)


DEFAULT_CFG = {
    "lhst_dt": "bfloat16",
    "rhs_dt": "bfloat16",
    "gp_split": 0,
    "act_split": 7,
    "pack_psum": True,
}


def kernel(features, dt, times_out, successor_kernel_ids, segment_ids_out,
           decay_rate, _bench=None, _cfg=None):
    import ml_dtypes

    cfg = dict(DEFAULT_CFG, **(_cfg or {}))
    features = np.asarray(features, dtype=np.float32)
    rate = _softplus(np.asarray(decay_rate, dtype=np.float32))

    featw, scal, iota, kpad = _preprocess(
        features, dt, times_out, successor_kernel_ids, segment_ids_out,
        decay_rate)
    if cfg["rhs_dt"] == "bfloat16":
        featw = featw.astype(ml_dtypes.bfloat16)

    nc = _build_program(kpad, rate, **cfg)

    const_rate = bool(np.ptp(rate) <= 1e-12 * max(1.0, abs(float(rate[0]))))
    in_maps = []
    for c in range(NCORES):
        m = {"featw": featw[c], "scal": scal[c], "iota": iota}
        if not const_rate:
            m["ratebc"] = np.tile(-rate, (128, 1)).astype(np.float32)
        in_maps.append(m)

    if _bench is not None:
        res = _run(nc, in_maps, **_bench)
        outs = [r["out"] for r in res.results]
        full = np.concatenate([o[:SLOTS_PER_CORE] for o in outs], axis=0)
        return full.reshape(N_OUT, K, C), res

    res = _run(nc, in_maps)
    outs = [r["out"] for r in res.results]
    full = np.concatenate([o[:SLOTS_PER_CORE] for o in outs], axis=0)
    return full.reshape(N_OUT, K, C)
